# revision 48
# baseline (speedup 1.0000x reference)
"""Trainium2 Bass kernel for the per-task embedding MLP (embedding_lookup).

Computation (per sample j with task t = task_ids[j]):
    h      = x[j] @ l1_emb[t].reshape(256, 128) + l1_bias[t]
    g      = gelu_exact(h)
    out[j] = sum(g * l2_emb[t]) + l2_bias[t, 0]

Strategy: shard the *task* axis across the 8 cores (125 contiguous tasks per
core), so each core streams a contiguous slab of l1_emb exactly once (the
memory roofline), instead of gathering a 128 KiB row per sample (4x more
traffic).  Samples are routed (host-side index math only) to the core owning
their task and packed into a fixed slot grid of W=12 columns per group
(tasks with more than W samples get extra groups with duplicated weight
rows), so all 8 cores run one identical SPMD program.

The stage-1 matmul operands (x, w1) are cast to fp16 on the host: fp32
matmuls on trn2 lower to LOW/HIGH double passes and fp16 halves the
dominant l1_emb DMA traffic.  PSUM accumulation and gelu stay fp32-ish;
measured end-to-end L2 relative error ~4e-4.

Trace-driven layout (final):
- Non-w1 traffic rides the scalar HWDGE ring as two merged DMAs (cbT =
  b1|w2 fp16, then x packed [128, 2*NSLOT]); no SWDGE anywhere, so the 8
  DMASW completion semaphores vanish from the teardown sweep.
- W=12 slot grid (vs 16) cuts the zero-padded x traffic 25% with no w1
  duplication in practice.
- Epilogue per block (emitted one block late so its PE reduce never
  head-of-line blocks the matmul stream): one fused DVE bias-add out of
  PSUM (fp16 out), gelu on ACT, w2-mult on DVE — all fp16 (2x DVE rate).
  NOTE: a DVE pre-load of the bias into PSUM with start=False matmuls on
  top is measurably faster but RACES on hardware (DVE-write vs PE
  accumulate visibility, ~40% of runs corrupt) — do not resurrect it.
- The hidden-dim reduction is transposed: per 128-slot chunk,
  LDWEIGHTS(prodt) @ ones[128,1] drops sum_h(g*w2) into a [128, GC] PSUM
  grid (each block owns 128-aligned columns; PE outputs must start at
  partition 0/32/64), so the final step is one 128-lane DVE copy to SBUF
  and one small output DMA.  l2_bias is added on the host.
- Few, fat blocks ([5,10,40,40,20,8,2]) with a shrinking tail: the
  scheduler's cross-engine stage lag makes the post-stream drain scale
  with block count, and the last block's epilogue chain is the critical
  path after the final w1 byte.
- Every PSUM tile is padded to a full 2 KB bank (accumulation-group state
  is per-bank; sharing a bank across blocks is a latent hazard).
"""

import numpy as np

import concourse.bacc as bacc
import concourse.mybir as mybir
import concourse.tile as tile
from concourse.bass_utils import run_bass_kernel_spmd

NUM_TASKS = 1000
N_FEATURES = 256
HIDDEN = 128
BATCH = 4096
N_CORES = 8
TPC = NUM_TASKS // N_CORES  # tasks per core = 125
GRP = 5                     # head block size / chunk granularity unit

# Module-level knobs for the test harness (the grader just calls kernel()).
MM_DTYPE = "float16"  # "float16" (fast path) or "float32" (exact fallback)
TRACE = False
TMPDIR = None  # optional fixed artifact dir for profiling runs
SIM_CORES = None  # e.g. [0]: run CoreSim for those cores instead of hardware
SIM_EXECUTOR_CLS = None  # optional InstructionExecutor subclass for CoreSim
LAST_RESULTS = None

_PROGRAM_CACHE = {}


def _block_sizes(W, NG):
    """Group counts per PSUM block.  A small first block so the first w1
    DMA's completion fires early; a shrinking tail ([13,5,2]) so the
    epilogue chain after the final w1 chunk lands is as short as possible."""
    GB = (512 // W // GRP) * GRP  # groups per PSUM block (GB*W <= 512)
    assert GB >= GRP
    head = [GRP, 2 * GRP]
    tail = [GRP, 1]
    rem = NG - sum(head) - sum(tail)
    assert rem > 0
    sizes = head + [GB] * (rem // GB) + ([rem % GB] if rem % GB else []) + tail
    assert sum(sizes) == NG and all(s <= GB for s in sizes)
    return sizes


def _build_program(W, NG, mm_dtype):
    """Emit the SPMD Tile program for slot width W and NG groups per core."""
    sizes = _block_sizes(W, NG)
    NSLOT = NG * W
    # Each block owns its own 128-aligned column range of the output grid
    # (PE outputs must start at partition 0), host unscrambles.
    colbase = np.cumsum([0] + [-(-(s * W) // 128) for s in sizes]).tolist()
    GC = colbase[-1]  # output grid columns
    f32 = mybir.dt.float32
    mdt = getattr(mybir.dt, mm_dtype)

    nc = bacc.Bacc("TRN2", target_bir_lowering=False, debug=False)

    xT_d = nc.dram_tensor("xTp", [128, 2 * NSLOT], mdt, kind="ExternalInput").ap()
    # w1 slab, host-packed per block in partition-major [128, gbt, 2, 128]
    # layout, one contiguous region per block (chunked DMAs each)
    w1_d = nc.dram_tensor(
        "w1s", [NG * N_FEATURES * HIDDEN], mdt, kind="ExternalInput"
    ).ap()
    # b1 and w2 (both fp16), packed side by side: one DMA.
    cb_d = nc.dram_tensor("cbT", [128, 2 * NG], mdt, kind="ExternalInput").ap()
    out_d = nc.dram_tensor("out", [128, GC], f32, kind="ExternalOutput").ap()

    with tile.TileContext(nc) as tc:
        with (
            tc.tile_pool(name="const", bufs=1) as constp,
            tc.tile_pool(name="w1pool", bufs=5) as w1p,
            tc.tile_pool(name="work", bufs=3) as workp,
            tc.tile_pool(name="prodp", bufs=len(sizes)) as prodp,
            # 6 bufs (not 7): leaving one block's worth of PSUM-recycle
            # pressure forces the scheduler to interleave epilogues with
            # the stream — with 7 independent banks it defers them all
            # past the stream and the drain serializes (+4 us measured).
            tc.tile_pool(name="hpsum", bufs=6, space="PSUM") as hpsp,
            tc.tile_pool(name="opsum", bufs=1, space="PSUM") as opsp,
        ):
            # consts first on the scalar ring (64 KB, lands early: block 0's
            # epilogue bias-add needs it), then the x columns (one DMA).
            cbT = constp.tile([128, 2 * NG], mdt)
            nc.scalar.dma_start(out=cbT, in_=cb_d)
            xc = constp.tile([128, 2 * NSLOT], mdt)
            nc.scalar.dma_start(out=xc, in_=xT_d)
            xc0 = xc[:, :NSLOT]
            xc1 = xc[:, NSLOT:]

            # ones-vector: rhs of the transposed hidden-dim reduce
            cones = constp.tile([128, 1], mdt)
            nc.vector.memset(cones, 1.0)

            out_sb = constp.tile([128, GC], f32)
            # single output PSUM grid, written piecewise by the reduces
            # (zeroed once: ragged chunks leave partition tails untouched).
            # Padded to a full 2 KB PSUM bank: PSUM accumulation-group state
            # is per-bank, and a DVE write into a bank that another block's
            # open matmul group owns corrupts results (timing-dependent) —
            # every PSUM tile here gets a private bank.
            ops = opsp.tile([128, 512], f32)
            nc.vector.memset(ops, 0.0)

            w1off = 0
            pending = None  # delayed epilogue of the previous block

            prods = []  # (b, cols, prodt): reduces run after the last matmul

            def epilogue_for(b, g0, gbt, ps):
                cols = gbt * W
                hs = workp.tile([128, cols], mdt, tag="hs")
                esb = workp.tile([128, cols], mdt, tag="esb")
                prodt = prodp.tile([128, cols], mdt, tag="prodt")
                halves = (
                    [(0, gbt // 2), (gbt // 2, gbt)] if gbt > GRP else [(0, gbt)]
                )
                for ga, gz in halves:
                    hsl = slice(ga * W, gz * W)
                    n_g = gz - ga
                    # hs = h + b1 (column-broadcast) out of PSUM, fp16 out
                    b1v = (
                        cbT[:, g0 + ga:g0 + gz]
                        .unsqueeze(2).broadcast_to([128, n_g, W])
                    )
                    nc.vector.scalar_tensor_tensor(
                        hs[:, hsl].rearrange("p (g w) -> p g w", w=W),
                        ps[:, hsl].rearrange("p (g w) -> p g w", w=W),
                        1.0, b1v,
                        op0=mybir.AluOpType.mult, op1=mybir.AluOpType.add,
                    )
                    nc.scalar.activation(
                        esb[:, hsl], hs[:, hsl],
                        mybir.ActivationFunctionType.Gelu,
                    )
                    # prod = g * w2 (column-broadcast), all fp16
                    w2v = (
                        cbT[:, NG + g0 + ga:NG + g0 + gz]
                        .unsqueeze(2).broadcast_to([128, n_g, W])
                    )
                    nc.vector.tensor_mul(
                        prodt[:, hsl].rearrange("p (g w) -> p g w", w=W),
                        esb[:, hsl].rearrange("p (g w) -> p g w", w=W),
                        w2v,
                    )
                prods.append((b, cols, prodt))

            for b, gbt in enumerate(sizes):
                g0 = sum(sizes[:b])
                cols = gbt * W
                base = g0 * W

                # full-bank tile (see ops above); only [:, :cols] is used
                pst = hpsp.tile([128, 512], f32, tag="hps")
                ps = pst[:, :cols]

                ln = 128 * gbt * 2 * 128
                w1t = w1p.tile([128, gbt, 2, 128], mdt, tag="w1t")
                blk = w1_d[w1off:w1off + ln].rearrange(
                    "(p g c h) -> p g c h", p=128, g=gbt, c=2
                )
                # Sub-DMAs with fine-grained completion sems (subtile deps
                # let matmuls start as each chunk lands).  2*GRP-task chunks
                # (5 KB per-partition lines, near the DMA knee) keep the
                # single sync ring streaming at line rate; odd-size blocks
                # go as one chunk.
                step = 2 * GRP if gbt % (2 * GRP) == 0 else gbt
                for q in range(gbt // step):
                    qs = slice(q * step, (q + 1) * step)
                    nc.sync.dma_start(out=w1t[:, qs], in_=blk[:, qs])
                w1off += ln
                for jj in range(gbt):
                    sl = slice(jj * W, (jj + 1) * W)
                    xsl = slice(base + jj * W, base + (jj + 1) * W)
                    nc.tensor.matmul(
                        ps[:, sl], lhsT=w1t[:, jj, 0], rhs=xc0[:, xsl],
                        start=True, stop=False,
                    )
                    nc.tensor.matmul(
                        ps[:, sl], lhsT=w1t[:, jj, 1], rhs=xc1[:, xsl],
                        start=False, stop=True,
                    )
                # The previous block's epilogue is emitted only now, after
                # this block's matmuls, so its reduce work never delays the
                # matmul stream.
                if pending is not None:
                    epilogue_for(*pending)
                pending = (b, g0, gbt, ps)

            epilogue_for(*pending)
            # All transposed reduces AFTER the last stage-1 matmul, so the
            # PE FIFO streams every w1 matmul uninterrupted and the tail
            # blocks' epilogues pipeline across ACT/DVE without each one
            # stalling the next block's matmuls.  Per 128-slot chunk of
            # block b: ops[0:m, colbase[b]+k] = prodt_chunk.T @ ones.
            for b, cols, prodt in prods:
                for k in range(-(-cols // 128)):
                    o0 = 128 * k
                    m = min(cols - o0, 128)
                    c = colbase[b] + k
                    nc.tensor.matmul(
                        ops[0:m, c:c + 1],
                        lhsT=prodt[:, o0:o0 + m], rhs=cones,
                        start=True, stop=True,
                    )
            # Bulk of the output (every block but the last — they own
            # disjoint 128-aligned columns) is copied out of PSUM and
            # DMA'd first; only a [128,1] copy + tiny DMA depend on the
            # final block's reduce.  (DVE copies: an ACT Copy would drag
            # in a second 1.3 us ACT table load.)
            cb_last = colbase[-2]
            nc.vector.tensor_scalar_mul(
                out_sb[:, :cb_last], ops[:, :cb_last], 1.0
            )
            nc.scalar.dma_start(
                out=out_d[:, :cb_last], in_=out_sb[:, :cb_last]
            )
            nc.vector.tensor_scalar_mul(
                out_sb[:, cb_last:GC], ops[:, cb_last:GC], 1.0
            )
            nc.sync.dma_start(out=out_d[:, cb_last:], in_=out_sb[:, cb_last:])

    nc.compile()
    return nc


def _get_program(W, NG, mm_dtype):
    key = (W, NG, mm_dtype)
    if key not in _PROGRAM_CACHE:
        _PROGRAM_CACHE[key] = _build_program(W, NG, mm_dtype)
    return _PROGRAM_CACHE[key]


def kernel(x, task_ids, l1_emb, l1_bias, l2_emb, l2_bias):
    global LAST_RESULTS
    x = np.ascontiguousarray(np.asarray(x, dtype=np.float32))
    tid = np.asarray(task_ids).astype(np.int64)
    l1_emb = np.ascontiguousarray(np.asarray(l1_emb, dtype=np.float32))
    l1_bias = np.ascontiguousarray(np.asarray(l1_bias, dtype=np.float32))
    l2_emb = np.ascontiguousarray(np.asarray(l2_emb, dtype=np.float32))
    l2_bias = np.ascontiguousarray(np.asarray(l2_bias, dtype=np.float32))

    B = x.shape[0]
    assert x.shape == (BATCH, N_FEATURES) and tid.shape == (BATCH,)

    mdt = np.float16 if MM_DTYPE == "float16" else np.float32
    W = 12

    # A "group" is (task, slice of up to W of its samples).  Tasks with more
    # than W samples get several groups (their w1 row is duplicated in the
    # slab); tasks with no samples still get one group so that in the common
    # case the slab is exactly the core's contiguous l1_emb slice.
    counts = np.bincount(tid, minlength=NUM_TASKS)
    ngroups = np.maximum(1, -(-counts // W)).astype(np.int64)  # per task
    ng_core = ngroups.reshape(N_CORES, TPC).sum(axis=1)
    NG = -(-int(ng_core.max()) // GRP) * GRP  # round up to a GRP multiple
    NSLOT = NG * W

    # per-slot position in the [128, GC] output grid (must mirror
    # _build_program's per-block 128-aligned column ranges)
    sizes = _block_sizes(W, NG)
    colbase = np.cumsum([0] + [-(-(s * W) // 128) for s in sizes])
    GC = int(colbase[-1])
    slot_p = np.empty(NSLOT, dtype=np.int64)
    slot_c = np.empty(NSLOT, dtype=np.int64)
    base = 0
    for b, gbt in enumerate(sizes):
        cols = gbt * W
        o = np.arange(cols)
        slot_p[base:base + cols] = o % 128
        slot_c[base:base + cols] = colbase[b] + o // 128
        base += cols

    # within-core group base of each task
    gbase = np.empty(NUM_TASKS, dtype=np.int64)
    for c in range(N_CORES):
        sl = slice(c * TPC, (c + 1) * TPC)
        cs = np.cumsum(ngroups[sl])
        gbase[sl] = cs - ngroups[sl]

    # slot routing: sample j -> (core, slot)
    order = np.argsort(tid, kind="stable")
    sorted_tid = tid[order]
    starts = np.flatnonzero(np.r_[True, np.diff(sorted_tid) != 0])
    run_len = np.diff(np.r_[starts, B])
    run_pos = np.arange(B) - np.repeat(starts, run_len)
    occ = np.empty(B, dtype=np.int64)
    occ[order] = run_pos
    core = tid // TPC
    slot = (gbase[tid] + occ // W) * W + occ % W

    # scatter x into per-core transposed, padded slot grids
    xT = np.zeros((N_CORES, N_FEATURES, NSLOT), dtype=mdt)
    xT[core, :, slot] = x.astype(mdt)

    in_maps = []
    for c in range(N_CORES):
        t0 = c * TPC
        sl = slice(t0, t0 + TPC)
        # task id of each group (padded to NG with the core's first task)
        gtask = np.repeat(np.arange(t0, t0 + TPC), ngroups[sl])
        if len(gtask) < NG:
            gtask = np.r_[gtask, np.full(NG - len(gtask), t0)]
        rows = l1_emb[gtask]  # [NG, 32768]
        # pack w1 per block: [gbt, 2, 128, 128] -> [128, gbt, 2, 128] flat
        parts = []
        cum = 0
        for gbt in sizes:
            blk = rows[cum:cum + gbt]
            blk = blk.reshape(gbt, 2, 128, 128).transpose(2, 0, 1, 3)
            parts.append(blk.astype(mdt).reshape(-1))
            cum += gbt
        xTc = xT[c].reshape(2, 128, NSLOT)
        in_maps.append({
            "xTp": np.ascontiguousarray(
                np.concatenate([xTc[0], xTc[1]], axis=1)
            ),
            "w1s": np.concatenate(parts),
            "cbT": np.ascontiguousarray(np.concatenate(
                [l1_bias[gtask].T, l2_emb[gtask].T], axis=1
            ).astype(mdt)),
        })

    nc = _get_program(W, NG, MM_DTYPE)
    if SIM_CORES is not None:
        from concourse.bass_interp import CoreSim

        sim_results = []
        for c in range(N_CORES):
            if c in SIM_CORES:
                kw = {}
                if SIM_EXECUTOR_CLS is not None:
                    kw["executor_cls"] = SIM_EXECUTOR_CLS
                sim = CoreSim(nc, publish_trace=False, **kw)
                for k, v in in_maps[c].items():
                    sim.tensor(k)[:] = v
                sim.simulate()
                sim_results.append({"out": np.array(sim.tensor("out"))})
            else:
                sim_results.append({"out": np.zeros((128, GC), np.float32)})
        outs = np.stack([r["out"].reshape(128, GC) for r in sim_results])
    else:
        res = run_bass_kernel_spmd(
            nc, in_maps, core_ids=list(range(N_CORES)), trace=TRACE,
            tmpdir=TMPDIR,
        )
        LAST_RESULTS = res
        outs = np.stack([r["out"].reshape(128, GC) for r in res.results])

    logits = outs[core, slot_p[slot], slot_c[slot]] + l2_bias[tid, 0]
    return logits[:, None].astype(np.float32)


# revision 49
# speedup vs baseline: 1.0811x; 1.0811x over previous
"""Trainium2 Bass kernel for the per-task embedding MLP (embedding_lookup).

Computation (per sample j with task t = task_ids[j]):
    h      = x[j] @ l1_emb[t].reshape(256, 128) + l1_bias[t]
    g      = gelu_exact(h)
    out[j] = sum(g * l2_emb[t]) + l2_bias[t, 0]

Strategy: shard the *task* axis across the 8 cores (125 contiguous tasks per
core), so each core streams a contiguous slab of l1_emb exactly once (the
memory roofline), instead of gathering a 128 KiB row per sample (4x more
traffic).  Samples are routed (host-side index math only) to the core owning
their task and packed into a fixed slot grid of W=12 columns per group
(tasks with more than W samples get extra groups with duplicated weight
rows), so all 8 cores run one identical SPMD program.

The stage-1 matmul operands (x, w1) are cast to fp16 on the host: fp32
matmuls on trn2 lower to LOW/HIGH double passes and fp16 halves the
dominant l1_emb DMA traffic.  PSUM accumulation and gelu stay fp32-ish;
measured end-to-end L2 relative error ~4e-4.

Trace-driven layout (final):
- Non-w1 traffic rides the scalar HWDGE ring as two merged DMAs (cbT =
  b1|w2 fp16, then x packed [128, 2*NSLOT]); no SWDGE anywhere, so the 8
  DMASW completion semaphores vanish from the teardown sweep.
- W=12 slot grid (vs 16) cuts the zero-padded x traffic 25% with no w1
  duplication in practice.
- Epilogue per block (emitted one block late so its PE reduce never
  head-of-line blocks the matmul stream): one fused DVE bias-add out of
  PSUM (fp16 out), gelu on ACT, w2-mult on DVE — all fp16 (2x DVE rate).
  NOTE: a DVE pre-load of the bias into PSUM with start=False matmuls on
  top is measurably faster but RACES on hardware (DVE-write vs PE
  accumulate visibility, ~40% of runs corrupt) — do not resurrect it.
- The hidden-dim reduction is transposed: per 128-slot chunk,
  LDWEIGHTS(prodt) @ ones[128,1] drops sum_h(g*w2) into a [128, GC] PSUM
  grid (each block owns 128-aligned columns; PE outputs must start at
  partition 0/32/64), so the final step is one 128-lane DVE copy to SBUF
  and one small output DMA.  l2_bias is added on the host.
- Few, fat blocks ([5,10,40,40,20,8,2]) with a shrinking tail: the
  scheduler's cross-engine stage lag makes the post-stream drain scale
  with block count, and the last block's epilogue chain is the critical
  path after the final w1 byte.
- Every PSUM tile is padded to a full 2 KB bank (accumulation-group state
  is per-bank; sharing a bank across blocks is a latent hazard).
"""

import numpy as np

import concourse.bacc as bacc
import concourse.mybir as mybir
import concourse.tile as tile
from concourse.bass_utils import run_bass_kernel_spmd

NUM_TASKS = 1000
N_FEATURES = 256
HIDDEN = 128
BATCH = 4096
N_CORES = 8
TPC = NUM_TASKS // N_CORES  # tasks per core = 125
GRP = 5                     # head block size / chunk granularity unit

# Module-level knobs for the test harness (the grader just calls kernel()).
MM_DTYPE = "float16"  # "float16" (fast path) or "float32" (exact fallback)
TRACE = False
TMPDIR = None  # optional fixed artifact dir for profiling runs
SIM_CORES = None  # e.g. [0]: run CoreSim for those cores instead of hardware
SIM_EXECUTOR_CLS = None  # optional InstructionExecutor subclass for CoreSim
LAST_RESULTS = None

_PROGRAM_CACHE = {}


def _block_sizes(W, NG):
    """Group counts per PSUM block.  A small first block so the first w1
    DMA's completion fires early; a shrinking tail ([13,5,2]) so the
    epilogue chain after the final w1 chunk lands is as short as possible."""
    GB = (512 // W // GRP) * GRP  # groups per PSUM block (GB*W <= 512)
    assert GB >= GRP
    head = [GRP, 2 * GRP]
    tail = [GRP, 1]
    rem = NG - sum(head) - sum(tail)
    assert rem > 0
    sizes = head + [GB] * (rem // GB) + ([rem % GB] if rem % GB else []) + tail
    assert sum(sizes) == NG and all(s <= GB for s in sizes)
    return sizes


def _build_program(W, NG, mm_dtype):
    """Emit the SPMD Tile program for slot width W and NG groups per core."""
    sizes = _block_sizes(W, NG)
    NSLOT = NG * W
    # Each block owns its own 128-aligned column range of the output grid
    # (PE outputs must start at partition 0), host unscrambles.
    colbase = np.cumsum([0] + [-(-(s * W) // 128) for s in sizes]).tolist()
    GC = colbase[-1]  # output grid columns
    f32 = mybir.dt.float32
    mdt = getattr(mybir.dt, mm_dtype)

    nc = bacc.Bacc("TRN2", target_bir_lowering=False, debug=False)

    xT_d = nc.dram_tensor("xTp", [128, 2 * NSLOT], mdt, kind="ExternalInput").ap()
    # w1 slab, host-packed per block in partition-major [128, gbt, 2, 128]
    # layout, one contiguous region per block (chunked DMAs each)
    w1_d = nc.dram_tensor(
        "w1s", [NG * N_FEATURES * HIDDEN], mdt, kind="ExternalInput"
    ).ap()
    # b1 and w2 (both fp16), packed side by side: one DMA.
    cb_d = nc.dram_tensor("cbT", [128, 2 * NG], mdt, kind="ExternalInput").ap()
    out_d = nc.dram_tensor("out", [128, GC], f32, kind="ExternalOutput").ap()

    with tile.TileContext(nc) as tc:
        with (
            tc.tile_pool(name="const", bufs=1) as constp,
            tc.tile_pool(name="w1pool", bufs=4) as w1p,
            tc.tile_pool(name="work", bufs=3) as workp,
            tc.tile_pool(name="prodp", bufs=len(sizes)) as prodp,
            tc.tile_pool(name="hpsum", bufs=6, space="PSUM") as hpsp,
            tc.tile_pool(name="opsum", bufs=1, space="PSUM") as opsp,
        ):
            # consts first on the scalar ring (64 KB, lands early: block 0's
            # epilogue bias-add needs it), then the x columns (one DMA).
            cbT = constp.tile([128, 2 * NG], mdt)
            nc.scalar.dma_start(out=cbT, in_=cb_d)
            xc = constp.tile([128, 2 * NSLOT], mdt)
            nc.scalar.dma_start(out=xc, in_=xT_d)
            xc0 = xc[:, :NSLOT]
            xc1 = xc[:, NSLOT:]

            # ones-vector: rhs of the transposed hidden-dim reduce
            cones = constp.tile([128, 1], mdt)
            nc.vector.memset(cones, 1.0)

            out_sb = constp.tile([128, GC], f32)
            # single output PSUM grid, written piecewise by the reduces
            # (zeroed once: ragged chunks leave partition tails untouched).
            # Padded to a full 2 KB PSUM bank: PSUM accumulation-group state
            # is per-bank, and a DVE write into a bank that another block's
            # open matmul group owns corrupts results (timing-dependent) —
            # every PSUM tile here gets a private bank.
            ops = opsp.tile([128, 512], f32)
            nc.vector.memset(ops, 0.0)

            w1off = 0
            pending = None  # delayed epilogue of the previous block

            prods = []  # (b, cols, prodt): reduces run after the last matmul

            def epilogue_for(b, g0, gbt, ps):
                cols = gbt * W
                hs = workp.tile([128, cols], mdt, tag="hs")
                esb = workp.tile([128, cols], mdt, tag="esb")
                prodt = prodp.tile([128, cols], mdt, tag="prodt")
                halves = (
                    [(0, gbt // 2), (gbt // 2, gbt)] if gbt > GRP else [(0, gbt)]
                )
                for ga, gz in halves:
                    hsl = slice(ga * W, gz * W)
                    n_g = gz - ga
                    # hs = h + b1 (column-broadcast) out of PSUM, fp16 out
                    b1v = (
                        cbT[:, g0 + ga:g0 + gz]
                        .unsqueeze(2).broadcast_to([128, n_g, W])
                    )
                    nc.vector.scalar_tensor_tensor(
                        hs[:, hsl].rearrange("p (g w) -> p g w", w=W),
                        ps[:, hsl].rearrange("p (g w) -> p g w", w=W),
                        1.0, b1v,
                        op0=mybir.AluOpType.mult, op1=mybir.AluOpType.add,
                    )
                    nc.scalar.activation(
                        esb[:, hsl], hs[:, hsl],
                        mybir.ActivationFunctionType.Gelu,
                    )
                    # prod = g * w2 (column-broadcast), all fp16
                    w2v = (
                        cbT[:, NG + g0 + ga:NG + g0 + gz]
                        .unsqueeze(2).broadcast_to([128, n_g, W])
                    )
                    nc.vector.tensor_mul(
                        prodt[:, hsl].rearrange("p (g w) -> p g w", w=W),
                        esb[:, hsl].rearrange("p (g w) -> p g w", w=W),
                        w2v,
                    )
                prods.append((b, cols, prodt))

            for b, gbt in enumerate(sizes):
                g0 = sum(sizes[:b])
                cols = gbt * W
                base = g0 * W

                # full-bank tile (see ops above); only [:, :cols] is used
                pst = hpsp.tile([128, 512], f32, tag="hps")
                ps = pst[:, :cols]

                ln = 128 * gbt * 2 * 128
                w1t = w1p.tile([128, gbt, 2, 128], mdt, tag="w1t")
                blk = w1_d[w1off:w1off + ln].rearrange(
                    "(p g c h) -> p g c h", p=128, g=gbt, c=2
                )
                # Sub-DMAs with fine-grained completion sems (subtile deps
                # let matmuls start as each chunk lands).  2*GRP-task chunks
                # (5 KB per-partition lines, near the DMA knee) keep the
                # single sync ring streaming at line rate; odd-size blocks
                # go as one chunk.
                step = 2 * GRP if gbt % (2 * GRP) == 0 else gbt
                for q in range(gbt // step):
                    qs = slice(q * step, (q + 1) * step)
                    nc.sync.dma_start(out=w1t[:, qs], in_=blk[:, qs])
                w1off += ln
                for jj in range(gbt):
                    sl = slice(jj * W, (jj + 1) * W)
                    xsl = slice(base + jj * W, base + (jj + 1) * W)
                    nc.tensor.matmul(
                        ps[:, sl], lhsT=w1t[:, jj, 0], rhs=xc0[:, xsl],
                        start=True, stop=False,
                    )
                    nc.tensor.matmul(
                        ps[:, sl], lhsT=w1t[:, jj, 1], rhs=xc1[:, xsl],
                        start=False, stop=True,
                    )
                # The previous block's epilogue is emitted only now, after
                # this block's matmuls, so its reduce work never delays the
                # matmul stream.
                if pending is not None:
                    epilogue_for(*pending)
                pending = (b, g0, gbt, ps)

            epilogue_for(*pending)
            # All transposed reduces AFTER the last stage-1 matmul, so the
            # PE FIFO streams every w1 matmul uninterrupted and the tail
            # blocks' epilogues pipeline across ACT/DVE without each one
            # stalling the next block's matmuls.  Per 128-slot chunk of
            # block b: ops[0:m, colbase[b]+k] = prodt_chunk.T @ ones.
            for b, cols, prodt in prods:
                for k in range(-(-cols // 128)):
                    o0 = 128 * k
                    m = min(cols - o0, 128)
                    c = colbase[b] + k
                    nc.tensor.matmul(
                        ops[0:m, c:c + 1],
                        lhsT=prodt[:, o0:o0 + m], rhs=cones,
                        start=True, stop=True,
                    )
            # Bulk of the output (every block but the last — they own
            # disjoint 128-aligned columns) is copied out of PSUM and
            # DMA'd first; only a [128,1] copy + tiny DMA depend on the
            # final block's reduce.  (DVE copies: an ACT Copy would drag
            # in a second 1.3 us ACT table load.)
            cb_last = colbase[-2]
            nc.vector.tensor_scalar_mul(
                out_sb[:, :cb_last], ops[:, :cb_last], 1.0
            )
            nc.scalar.dma_start(
                out=out_d[:, :cb_last], in_=out_sb[:, :cb_last]
            )
            nc.vector.tensor_scalar_mul(
                out_sb[:, cb_last:GC], ops[:, cb_last:GC], 1.0
            )
            nc.sync.dma_start(out=out_d[:, cb_last:], in_=out_sb[:, cb_last:])

    nc.compile()
    return nc


def _get_program(W, NG, mm_dtype):
    key = (W, NG, mm_dtype)
    if key not in _PROGRAM_CACHE:
        _PROGRAM_CACHE[key] = _build_program(W, NG, mm_dtype)
    return _PROGRAM_CACHE[key]


def kernel(x, task_ids, l1_emb, l1_bias, l2_emb, l2_bias):
    global LAST_RESULTS
    x = np.ascontiguousarray(np.asarray(x, dtype=np.float32))
    tid = np.asarray(task_ids).astype(np.int64)
    l1_emb = np.ascontiguousarray(np.asarray(l1_emb, dtype=np.float32))
    l1_bias = np.ascontiguousarray(np.asarray(l1_bias, dtype=np.float32))
    l2_emb = np.ascontiguousarray(np.asarray(l2_emb, dtype=np.float32))
    l2_bias = np.ascontiguousarray(np.asarray(l2_bias, dtype=np.float32))

    B = x.shape[0]
    assert x.shape == (BATCH, N_FEATURES) and tid.shape == (BATCH,)

    mdt = np.float16 if MM_DTYPE == "float16" else np.float32
    W = 12

    # A "group" is (task, slice of up to W of its samples).  Tasks with more
    # than W samples get several groups (their w1 row is duplicated in the
    # slab); tasks with no samples still get one group so that in the common
    # case the slab is exactly the core's contiguous l1_emb slice.
    counts = np.bincount(tid, minlength=NUM_TASKS)
    ngroups = np.maximum(1, -(-counts // W)).astype(np.int64)  # per task
    ng_core = ngroups.reshape(N_CORES, TPC).sum(axis=1)
    NG = -(-int(ng_core.max()) // GRP) * GRP  # round up to a GRP multiple
    NSLOT = NG * W

    # per-slot position in the [128, GC] output grid (must mirror
    # _build_program's per-block 128-aligned column ranges)
    sizes = _block_sizes(W, NG)
    colbase = np.cumsum([0] + [-(-(s * W) // 128) for s in sizes])
    GC = int(colbase[-1])
    slot_p = np.empty(NSLOT, dtype=np.int64)
    slot_c = np.empty(NSLOT, dtype=np.int64)
    base = 0
    for b, gbt in enumerate(sizes):
        cols = gbt * W
        o = np.arange(cols)
        slot_p[base:base + cols] = o % 128
        slot_c[base:base + cols] = colbase[b] + o // 128
        base += cols

    # within-core group base of each task
    gbase = np.empty(NUM_TASKS, dtype=np.int64)
    for c in range(N_CORES):
        sl = slice(c * TPC, (c + 1) * TPC)
        cs = np.cumsum(ngroups[sl])
        gbase[sl] = cs - ngroups[sl]

    # slot routing: sample j -> (core, slot)
    order = np.argsort(tid, kind="stable")
    sorted_tid = tid[order]
    starts = np.flatnonzero(np.r_[True, np.diff(sorted_tid) != 0])
    run_len = np.diff(np.r_[starts, B])
    run_pos = np.arange(B) - np.repeat(starts, run_len)
    occ = np.empty(B, dtype=np.int64)
    occ[order] = run_pos
    core = tid // TPC
    slot = (gbase[tid] + occ // W) * W + occ % W

    # scatter x into per-core transposed, padded slot grids
    xT = np.zeros((N_CORES, N_FEATURES, NSLOT), dtype=mdt)
    xT[core, :, slot] = x.astype(mdt)

    in_maps = []
    for c in range(N_CORES):
        t0 = c * TPC
        sl = slice(t0, t0 + TPC)
        # task id of each group (padded to NG with the core's first task)
        gtask = np.repeat(np.arange(t0, t0 + TPC), ngroups[sl])
        if len(gtask) < NG:
            gtask = np.r_[gtask, np.full(NG - len(gtask), t0)]
        rows = l1_emb[gtask]  # [NG, 32768]
        # pack w1 per block: [gbt, 2, 128, 128] -> [128, gbt, 2, 128] flat
        parts = []
        cum = 0
        for gbt in sizes:
            blk = rows[cum:cum + gbt]
            blk = blk.reshape(gbt, 2, 128, 128).transpose(2, 0, 1, 3)
            parts.append(blk.astype(mdt).reshape(-1))
            cum += gbt
        xTc = xT[c].reshape(2, 128, NSLOT)
        in_maps.append({
            "xTp": np.ascontiguousarray(
                np.concatenate([xTc[0], xTc[1]], axis=1)
            ),
            "w1s": np.concatenate(parts),
            "cbT": np.ascontiguousarray(np.concatenate(
                [l1_bias[gtask].T, l2_emb[gtask].T], axis=1
            ).astype(mdt)),
        })

    nc = _get_program(W, NG, MM_DTYPE)
    if SIM_CORES is not None:
        from concourse.bass_interp import CoreSim

        sim_results = []
        for c in range(N_CORES):
            if c in SIM_CORES:
                kw = {}
                if SIM_EXECUTOR_CLS is not None:
                    kw["executor_cls"] = SIM_EXECUTOR_CLS
                sim = CoreSim(nc, publish_trace=False, **kw)
                for k, v in in_maps[c].items():
                    sim.tensor(k)[:] = v
                sim.simulate()
                sim_results.append({"out": np.array(sim.tensor("out"))})
            else:
                sim_results.append({"out": np.zeros((128, GC), np.float32)})
        outs = np.stack([r["out"].reshape(128, GC) for r in sim_results])
    else:
        res = run_bass_kernel_spmd(
            nc, in_maps, core_ids=list(range(N_CORES)), trace=TRACE,
            tmpdir=TMPDIR,
        )
        LAST_RESULTS = res
        outs = np.stack([r["out"].reshape(128, GC) for r in res.results])

    logits = outs[core, slot_p[slot], slot_c[slot]] + l2_bias[tid, 0]
    return logits[:, None].astype(np.float32)
